# revision 1
# baseline (speedup 1.0000x reference)
"""FFT-based linear convolution of two 2^23-point real signals on 8 trn2 NeuronCores.

Math: conv(a, x) = Im(ifft(fft(a + i*x)^2)) / 2, with the 2^24-point FFT done as a
3-factor (256^3) matmul FFT. Stage A (over n1) is computed r-sharded across cores,
one AllToAll reshards to k1-sharded for the middle row-FFTs (stages B, C), the
pointwise square happens in the digit-reversed domain, then the inverse stages
(C', B') run locally, a second AllToAll reshards back, and inverse stage A'
produces only the imaginary part of the first half of the time-domain signal.
"""
import os
import numpy as np

os.environ.setdefault("JAX_PLATFORMS", "")
import jax

jax.config.update("jax_compilation_cache_dir", "/tmp/jax_neff_cache")
jax.config.update("jax_persistent_cache_min_entry_size_bytes", -1)
jax.config.update("jax_persistent_cache_min_compile_time_secs", 0)

import concourse.bass as bass
import concourse.tile as tile
from concourse import bacc, mybir
from concourse.bass_utils import run_bass_kernel_spmd

N = 8388608          # input length
M = 2 * N            # FFT size = 2^24
B = 256              # radix
R = B * B            # 65536
W = 8                # cores
RL = R // W          # 8192 columns of r per core
CH = 512             # free-dim chunk in stages A / A'
NCHUNK = RL // CH    # 16
F32 = mybir.dt.float32

# matmul operands use float32r (full-rate PE, ~13-bit mantissa). Operands must be
# produced by a rounding op: gpsimd cast-DMA for host data, f32r-output compute
# ops for device intermediates. Set USE_F32R=False to fall back to exact fp32.
USE_F32R = False
MMD = mybir.dt.float32r if USE_F32R else F32


def _mm(ap):
    return ap


def build_nc():
    nc = bacc.Bacc("TRN2", target_bir_lowering=False, debug=False, num_devices=W)

    a_in = nc.dram_tensor("a_c", [128, RL], MMD, kind="ExternalInput")
    x_in = nc.dram_tensor("x_c", [128, RL], MMD, kind="ExternalInput")
    t1r_in = nc.dram_tensor("t1r", [B, RL], F32, kind="ExternalInput")
    t1i_in = nc.dram_tensor("t1i", [B, RL], F32, kind="ExternalInput")
    dr_in = nc.dram_tensor("dr", [B, B], MMD, kind="ExternalInput")
    di_in = nc.dram_tensor("di", [B, B], MMD, kind="ExternalInput")
    ndi_in = nc.dram_tensor("ndi", [B, B], MMD, kind="ExternalInput")
    t2r_in = nc.dram_tensor("t2r", [B, B], F32, kind="ExternalInput")
    t2i_in = nc.dram_tensor("t2i", [B, B], F32, kind="ExternalInput")
    aw1_in = nc.dram_tensor("aw1", [B, 128], MMD, kind="ExternalInput")
    aw2_in = nc.dram_tensor("aw2", [B, 128], MMD, kind="ExternalInput")
    y_out = nc.dram_tensor("y_c", [128, RL], F32, kind="ExternalOutput")

    rg = [list(range(W))]

    with tile.TileContext(nc) as tc:
        with tc.tile_pool(name="dram", bufs=1, space="DRAM") as dram, \
             tc.tile_pool(name="consts", bufs=1) as consts:
            cc1_in = dram.tile([W, 32, 2, RL], MMD)
            cc1_out = dram.tile([W, 32, 2, RL], MMD)
            cc2_in = dram.tile([W, 32, 2, 32, B], F32)
            cc2_out = dram.tile([W, 32, 2, 32, B], F32)

            # ---- constant tables in SBUF ----
            # D row-halves (128, 256); col-slices give 128x128 blocks.
            dr_row, di_row, ndi_row, t2r_row, t2i_row = [], [], [], [], []
            for p in range(2):
                for lst, src, dt_ in ((dr_row, dr_in, MMD), (di_row, di_in, MMD), (ndi_row, ndi_in, MMD),
                                 (t2r_row, t2r_in, F32), (t2i_row, t2i_in, F32)):
                    t = consts.tile([128, B], dt_, name=f"c_{src.name}_{p}", tag=f"c_{src.name}_{p}")
                    nc.sync.dma_start(t[:], src[128 * p:128 * (p + 1), :])
                    lst.append(t)
            aw1_blk, aw2_blk = [], []
            for p in range(2):
                for lst, src in ((aw1_blk, aw1_in), (aw2_blk, aw2_in)):
                    t = consts.tile([128, 128], MMD, name=f"c_{src.name}_{p}", tag=f"c_{src.name}_{p}")
                    nc.sync.dma_start(t[:], src[128 * p:128 * (p + 1), :])
                    lst.append(t)

            # ================= Phase A: stage A + T1 twiddle =================
            with tc.tile_pool(name="a_io", bufs=1) as a_io, \
                 tc.tile_pool(name="a_t1", bufs=8) as a_t1, \
                 tc.tile_pool(name="a_tmp", bufs=16) as a_tmp, \
                 tc.tile_pool(name="a_out", bufs=6) as a_outp, \
                 tc.tile_pool(name="a_ps", bufs=4, space="PSUM") as a_ps:
                a_full = a_io.tile([128, RL], MMD)
                nc.sync.dma_start(a_full[:], a_in[:, :])
                x_full = a_io.tile([128, RL], MMD)
                nc.sync.dma_start(x_full[:], x_in[:, :])

                for c in range(NCHUNK):
                    a_sl = a_full[:, c * CH:(c + 1) * CH]
                    x_sl = x_full[:, c * CH:(c + 1) * CH]
                    for h in range(2):
                        hs = slice(128 * h, 128 * (h + 1))
                        ps_r = a_ps.tile([128, CH], F32, tag="ps")
                        ps_i = a_ps.tile([128, CH], F32, tag="ps")
                        nc.tensor.matmul(ps_r[:], _mm(dr_row[0][:, hs]), _mm(a_sl),
                                         start=True, stop=False)
                        nc.tensor.matmul(ps_i[:], _mm(dr_row[0][:, hs]), _mm(x_sl),
                                         start=True, stop=False)
                        nc.tensor.matmul(ps_r[:], _mm(ndi_row[0][:, hs]), _mm(x_sl),
                                         start=False, stop=True)
                        nc.tensor.matmul(ps_i[:], _mm(di_row[0][:, hs]), _mm(a_sl),
                                         start=False, stop=True)

                        t1r_t = a_t1.tile([128, CH], F32, tag="t1")
                        nc.sync.dma_start(t1r_t[:], t1r_in[hs, c * CH:(c + 1) * CH])
                        t1i_t = a_t1.tile([128, CH], F32, tag="t1")
                        nc.sync.dma_start(t1i_t[:], t1i_in[hs, c * CH:(c + 1) * CH])

                        # Y' = (ps_r + i ps_i) * (t1r + i t1i), packed [Re | Im]
                        out_t = a_outp.tile([128, 2 * CH], MMD, tag="aout")
                        m1 = a_tmp.tile([128, CH], F32, tag="tmp")
                        m2 = a_tmp.tile([128, CH], F32, tag="tmp")
                        m3 = a_tmp.tile([128, CH], F32, tag="tmp")
                        m4 = a_tmp.tile([128, CH], F32, tag="tmp")
                        nc.vector.tensor_mul(m1[:], ps_r[:], t1r_t[:])
                        nc.vector.tensor_mul(m2[:], ps_i[:], t1i_t[:])
                        nc.vector.tensor_mul(m3[:], ps_r[:], t1i_t[:])
                        nc.vector.tensor_mul(m4[:], ps_i[:], t1r_t[:])
                        nc.gpsimd.tensor_sub(out_t[:, 0:CH], m1[:], m2[:])
                        nc.gpsimd.tensor_add(out_t[:, CH:2 * CH], m3[:], m4[:])

                        # store to A2A-1 input: dims (j=4, k1l=32, plane=2, rl=512)
                        nc.sync.dma_start(
                            cc1_in[4 * h:4 * (h + 1), :, :, c * CH:(c + 1) * CH],
                            out_t[:])

            nc.gpsimd.collective_compute(
                "AllToAll", mybir.AluOpType.bypass, replica_groups=rg,
                ins=[cc1_in.opt()], outs=[cc1_out.opt()])

            # ================= Middle: per-k1 row FFT + square ================
            with tc.tile_pool(name="m_in", bufs=16) as m_in, \
                 tc.tile_pool(name="m_sb", bufs=16) as m_sb, \
                 tc.tile_pool(name="m_out", bufs=12) as m_out, \
                 tc.tile_pool(name="m_ps", bufs=8, space="PSUM") as m_ps:
                for k1l in range(32):
                    # load Y[k1] as (n2, n3), split in n2-halves, planes
                    y_t = []  # [n2h][plane]
                    for n2h in range(2):
                        row = []
                        for pl in range(2):
                            t = m_in.tile([128, B], MMD, tag="yin")
                            nc.sync.dma_start(
                                t[:], cc1_out[4 * n2h:4 * (n2h + 1), k1l, pl, :])
                            row.append(t)
                        y_t.append(row)

                    # stage B (data as weights): Z^T (n3, k2) in PSUM
                    zt_ps = []  # [n3h][plane]
                    for n3h in range(2):
                        ns = slice(128 * n3h, 128 * (n3h + 1))
                        zr = m_ps.tile([128, B], F32, tag="mps")
                        zi = m_ps.tile([128, B], F32, tag="mps")
                        for n2h in range(2):
                            st = n2h == 0
                            sp = n2h == 1
                            nc.tensor.matmul(zr[:], _mm(y_t[n2h][0][:, ns]), _mm(dr_row[n2h][:]),
                                             start=st, stop=False, skip_group_check=True)
                            nc.tensor.matmul(zi[:], _mm(y_t[n2h][0][:, ns]), _mm(di_row[n2h][:]),
                                             start=st, stop=False, skip_group_check=True)
                            nc.tensor.matmul(zr[:], _mm(y_t[n2h][1][:, ns]), _mm(ndi_row[n2h][:]),
                                             start=False, stop=sp, skip_group_check=True)
                            nc.tensor.matmul(zi[:], _mm(y_t[n2h][1][:, ns]), _mm(dr_row[n2h][:]),
                                             start=False, stop=sp, skip_group_check=True)
                        zt_ps.append((zr, zi))

                    # T2 twiddle (Z^T layout: mult by T2^T rows) -> SBUF
                    zt_sb = []
                    for n3h in range(2):
                        zr, zi = zt_ps[n3h]
                        or_ = m_sb.tile([128, B], MMD, tag="zt")
                        oi_ = m_sb.tile([128, B], MMD, tag="zt")
                        p1 = m_sb.tile([128, B], F32, tag="mtmp")
                        p2 = m_sb.tile([128, B], F32, tag="mtmp")
                        p3 = m_sb.tile([128, B], F32, tag="mtmp")
                        p4 = m_sb.tile([128, B], F32, tag="mtmp")
                        nc.vector.tensor_mul(p1[:], zr[:], t2r_row[n3h][:])
                        nc.vector.tensor_mul(p2[:], zi[:], t2i_row[n3h][:])
                        nc.vector.tensor_mul(p3[:], zr[:], t2i_row[n3h][:])
                        nc.vector.tensor_mul(p4[:], zi[:], t2r_row[n3h][:])
                        nc.gpsimd.tensor_sub(or_[:], p1[:], p2[:])
                        nc.gpsimd.tensor_add(oi_[:], p3[:], p4[:])
                        zt_sb.append((or_, oi_))

                    # stage C (DFT stationary): U^T (k3, k2) in PSUM
                    ut_ps = []
                    for k3h in range(2):
                        ks = slice(128 * k3h, 128 * (k3h + 1))
                        ur = m_ps.tile([128, B], F32, tag="mps")
                        ui = m_ps.tile([128, B], F32, tag="mps")
                        for n3h in range(2):
                            st = n3h == 0
                            sp = n3h == 1
                            nc.tensor.matmul(ur[:], _mm(dr_row[n3h][:, ks]), _mm(zt_sb[n3h][0][:]),
                                             start=st, stop=False, skip_group_check=True)
                            nc.tensor.matmul(ui[:], _mm(di_row[n3h][:, ks]), _mm(zt_sb[n3h][0][:]),
                                             start=st, stop=False, skip_group_check=True)
                            nc.tensor.matmul(ur[:], _mm(ndi_row[n3h][:, ks]), _mm(zt_sb[n3h][1][:]),
                                             start=False, stop=sp, skip_group_check=True)
                            nc.tensor.matmul(ui[:], _mm(dr_row[n3h][:, ks]), _mm(zt_sb[n3h][1][:]),
                                             start=False, stop=sp, skip_group_check=True)
                        ut_ps.append((ur, ui))

                    # square: S = U^2 (k3, k2) -> SBUF
                    s_sb = []
                    for k3h in range(2):
                        ur, ui = ut_ps[k3h]
                        sr = m_sb.tile([128, B], MMD, tag="ssb")
                        si = m_sb.tile([128, B], MMD, tag="ssb")
                        uc = m_sb.tile([128, B], F32, tag="mtmp")
                        q1 = m_sb.tile([128, B], F32, tag="mtmp")
                        q2 = m_sb.tile([128, B], F32, tag="mtmp")
                        # only one PSUM operand allowed per vector op: stage ur in SBUF
                        nc.scalar.copy(uc[:], ur[:])
                        nc.vector.tensor_add(q1[:], uc[:], ui[:])
                        nc.vector.tensor_sub(q2[:], uc[:], ui[:])
                        nc.vector.scalar_tensor_tensor(
                            si[:], uc[:], 2.0, ui[:],
                            mybir.AluOpType.mult, mybir.AluOpType.mult)
                        nc.gpsimd.tensor_mul(sr[:], q1[:], q2[:])
                        s_sb.append((sr, si))

                    # stage C' (data as weights): Z2 (k2, n3) in PSUM
                    z2_ps = []
                    for k2h in range(2):
                        ks = slice(128 * k2h, 128 * (k2h + 1))
                        zr = m_ps.tile([128, B], F32, tag="mps")
                        zi = m_ps.tile([128, B], F32, tag="mps")
                        for k3h in range(2):
                            st = k3h == 0
                            sp = k3h == 1
                            nc.tensor.matmul(zr[:], _mm(s_sb[k3h][0][:, ks]), _mm(dr_row[k3h][:]),
                                             start=st, stop=False, skip_group_check=True)
                            nc.tensor.matmul(zi[:], _mm(s_sb[k3h][0][:, ks]), _mm(ndi_row[k3h][:]),
                                             start=st, stop=False, skip_group_check=True)
                            nc.tensor.matmul(zr[:], _mm(s_sb[k3h][1][:, ks]), _mm(di_row[k3h][:]),
                                             start=False, stop=sp, skip_group_check=True)
                            nc.tensor.matmul(zi[:], _mm(s_sb[k3h][1][:, ks]), _mm(dr_row[k3h][:]),
                                             start=False, stop=sp, skip_group_check=True)
                        z2_ps.append((zr, zi))

                    # conj(T2) twiddle (natural (k2, n3) layout) -> SBUF
                    y2_sb = []
                    for k2h in range(2):
                        zr, zi = z2_ps[k2h]
                        or_ = m_sb.tile([128, B], MMD, tag="y2")
                        oi_ = m_sb.tile([128, B], MMD, tag="y2")
                        p1 = m_sb.tile([128, B], F32, tag="mtmp")
                        p2 = m_sb.tile([128, B], F32, tag="mtmp")
                        p3 = m_sb.tile([128, B], F32, tag="mtmp")
                        p4 = m_sb.tile([128, B], F32, tag="mtmp")
                        nc.vector.tensor_mul(p1[:], zr[:], t2r_row[k2h][:])
                        nc.vector.tensor_mul(p2[:], zi[:], t2i_row[k2h][:])
                        nc.vector.tensor_mul(p3[:], zi[:], t2r_row[k2h][:])
                        nc.vector.tensor_mul(p4[:], zr[:], t2i_row[k2h][:])
                        nc.gpsimd.tensor_add(or_[:], p1[:], p2[:])
                        nc.gpsimd.tensor_sub(oi_[:], p3[:], p4[:])
                        y2_sb.append((or_, oi_))

                    # stage B' (DFT stationary, conj D): Y' (n2, n3) in PSUM
                    for n2h in range(2):
                        ns = slice(128 * n2h, 128 * (n2h + 1))
                        yr = m_ps.tile([128, B], F32, tag="mps")
                        yi = m_ps.tile([128, B], F32, tag="mps")
                        for k2h in range(2):
                            st = k2h == 0
                            sp = k2h == 1
                            nc.tensor.matmul(yr[:], _mm(dr_row[k2h][:, ns]), _mm(y2_sb[k2h][0][:]),
                                             start=st, stop=False, skip_group_check=True)
                            nc.tensor.matmul(yi[:], _mm(dr_row[k2h][:, ns]), _mm(y2_sb[k2h][1][:]),
                                             start=st, stop=False, skip_group_check=True)
                            nc.tensor.matmul(yr[:], _mm(di_row[k2h][:, ns]), _mm(y2_sb[k2h][1][:]),
                                             start=False, stop=sp, skip_group_check=True)
                            nc.tensor.matmul(yi[:], _mm(ndi_row[k2h][:, ns]), _mm(y2_sb[k2h][0][:]),
                                             start=False, stop=sp, skip_group_check=True)
                        # copy to SBUF and store to A2A-2 input
                        for pl, ps in ((0, yr), (1, yi)):
                            o = m_out.tile([128, B], F32, tag="mout")
                            nc.scalar.copy(o[:], ps[:])
                            nc.sync.dma_start(
                                cc2_in[4 * n2h:4 * (n2h + 1), k1l, pl, :, :], o[:])

            nc.gpsimd.collective_compute(
                "AllToAll", mybir.AluOpType.bypass, replica_groups=rg,
                ins=[cc2_in.opt()], outs=[cc2_out.opt()])

            # ============ Phase A': conj(T1), inverse stage A (Im only) ============
            with tc.tile_pool(name="f_in", bufs=16) as f_in, \
                 tc.tile_pool(name="f_t1", bufs=8) as f_t1, \
                 tc.tile_pool(name="f_tmp", bufs=16) as f_tmp, \
                 tc.tile_pool(name="f_out", bufs=6) as f_outp, \
                 tc.tile_pool(name="f_ps", bufs=4, space="PSUM") as f_ps:
                for c in range(NCHUNK):
                    ps_o = f_ps.tile([128, CH], F32, tag="fps")
                    for h in range(2):
                        hs = slice(128 * h, 128 * (h + 1))
                        pr = f_in.tile([128, CH], F32, tag="pin")
                        nc.sync.dma_start(
                            pr[:], cc2_out[4 * h:4 * (h + 1), :, 0, 2 * c:2 * (c + 1), :])
                        pi = f_in.tile([128, CH], F32, tag="pin")
                        nc.sync.dma_start(
                            pi[:], cc2_out[4 * h:4 * (h + 1), :, 1, 2 * c:2 * (c + 1), :])
                        t1r_t = f_t1.tile([128, CH], F32, tag="ft1")
                        nc.sync.dma_start(t1r_t[:], t1r_in[hs, c * CH:(c + 1) * CH])
                        t1i_t = f_t1.tile([128, CH], F32, tag="ft1")
                        nc.sync.dma_start(t1i_t[:], t1i_in[hs, c * CH:(c + 1) * CH])

                        # Yf = P * conj(T1)
                        yfr = f_tmp.tile([128, CH], MMD, tag="yf")
                        yfi = f_tmp.tile([128, CH], MMD, tag="yf")
                        p1 = f_tmp.tile([128, CH], F32, tag="ftmp")
                        p2 = f_tmp.tile([128, CH], F32, tag="ftmp")
                        p3 = f_tmp.tile([128, CH], F32, tag="ftmp")
                        p4 = f_tmp.tile([128, CH], F32, tag="ftmp")
                        nc.vector.tensor_mul(p1[:], pr[:], t1r_t[:])
                        nc.gpsimd.tensor_mul(p2[:], pi[:], t1i_t[:])
                        nc.vector.tensor_mul(p3[:], pi[:], t1r_t[:])
                        nc.gpsimd.tensor_mul(p4[:], pr[:], t1i_t[:])
                        nc.vector.tensor_add(yfr[:], p1[:], p2[:])
                        nc.vector.tensor_sub(yfi[:], p3[:], p4[:])

                        st = h == 0
                        sp = h == 1
                        nc.tensor.matmul(ps_o[:], _mm(aw1_blk[h][:]), _mm(yfi[:]),
                                         start=st, stop=False, skip_group_check=True)
                        nc.tensor.matmul(ps_o[:], _mm(aw2_blk[h][:]), _mm(yfr[:]),
                                         start=False, stop=sp, skip_group_check=True)

                    o = f_outp.tile([128, CH], F32, tag="fout")
                    nc.scalar.copy(o[:], ps_o[:])
                    nc.sync.dma_start(y_out[:, c * CH:(c + 1) * CH], o[:])

    nc.compile()
    return nc


_NC = None
_TABLES = None


def _tables():
    global _TABLES
    if _TABLES is None:
        k = np.arange(B)
        D = np.exp(-2j * np.pi * np.outer(k, k) / B)
        T2 = np.exp(-2j * np.pi * np.outer(k, k) / R)
        s = 1.0 / (2.0 * M)
        dr = np.ascontiguousarray(D.real.astype(np.float32))
        di = np.ascontiguousarray(D.imag.astype(np.float32))
        t1s = []
        for c in range(W):
            r = np.arange(c * RL, (c + 1) * RL)
            T1 = np.exp(-2j * np.pi * np.outer(k, r) / M)
            t1s.append((np.ascontiguousarray(T1.real.astype(np.float32)),
                        np.ascontiguousarray(T1.imag.astype(np.float32))))
        _TABLES = dict(
            dr=dr, di=di, ndi=np.ascontiguousarray(-di),
            t2r=np.ascontiguousarray(T2.real.astype(np.float32)),
            t2i=np.ascontiguousarray(T2.imag.astype(np.float32)),
            aw1=np.ascontiguousarray((s * D.real[:, :128]).astype(np.float32)),
            aw2=np.ascontiguousarray((-s * D.imag[:, :128]).astype(np.float32)),
            t1s=t1s,
        )
    return _TABLES


def kernel(a, x, _want_trace=False, **_unused):
    global _NC
    a = np.asarray(a, dtype=np.float32)
    x = np.asarray(x, dtype=np.float32)
    tb = _tables()
    if _NC is None:
        _NC = build_nc()

    a3 = a.reshape(128, W, RL)
    x3 = x.reshape(128, W, RL)
    in_maps = []
    for c in range(W):
        in_maps.append(dict(
            a_c=np.ascontiguousarray(a3[:, c, :]),
            x_c=np.ascontiguousarray(x3[:, c, :]),
            t1r=tb["t1s"][c][0], t1i=tb["t1s"][c][1],
            dr=tb["dr"], di=tb["di"], ndi=tb["ndi"],
            t2r=tb["t2r"], t2i=tb["t2i"],
            aw1=tb["aw1"], aw2=tb["aw2"],
        ))
    res = run_bass_kernel_spmd(_NC, in_maps, core_ids=list(range(W)),
                               trace=_want_trace)
    full = np.empty((128, R), dtype=np.float32)
    for c in range(W):
        full[:, c * RL:(c + 1) * RL] = res.results[c]["y_c"]
    out = full.reshape(-1)
    if _want_trace:
        return out, res
    return out



# revision 6
# speedup vs baseline: 55.4089x; 55.4089x over previous
"""FFT-based linear convolution of two 2^23-point real signals on 8 trn2 NeuronCores.

Math: conv(a, x) = Im(ifft(fft(a + i*x)^2)) / 2, with the 2^24-point FFT done as a
3-factor (256^3) matmul FFT. Stage A (over n1) is computed r-sharded across cores,
one AllToAll reshards to k1-sharded for the middle row-FFTs (stages B, C), the
pointwise square happens in the digit-reversed domain, then the inverse stages
(C', B') run locally, a second AllToAll reshards back, and inverse stage A'
produces only the imaginary part of the first half of the time-domain signal.
"""
import os
import numpy as np

os.environ.setdefault("JAX_PLATFORMS", "")
import jax

jax.config.update("jax_compilation_cache_dir", "/tmp/jax_neff_cache")
jax.config.update("jax_persistent_cache_min_entry_size_bytes", -1)
jax.config.update("jax_persistent_cache_min_compile_time_secs", 0)

import concourse.bass as bass
import concourse.tile as tile
from concourse import bacc, mybir
from concourse.bass_utils import run_bass_kernel_spmd

N = 8388608          # input length
M = 2 * N            # FFT size = 2^24
B = 256              # radix
R = B * B            # 65536
W = 8                # cores
RL = R // W          # 8192 columns of r per core
CH = 512             # free-dim chunk in stages A / A'
NCHUNK = RL // CH    # 16
F32 = mybir.dt.float32

# matmul operands use float32r (full-rate PE, ~13-bit mantissa). Operands must be
# produced by a rounding op: gpsimd cast-DMA for host data, f32r-output compute
# ops for device intermediates. Set USE_F32R=False to fall back to exact fp32.
USE_F32R = True
MMD = mybir.dt.float32r if USE_F32R else F32


def _mm(ap):
    return ap


def build_nc(reps=1):
    """Build the kernel NEFF. reps>1 runs the full pipeline that many times
    back-to-back inside one NEFF (same inputs, same output) — used by test.py
    to measure per-execution device time as a dispatch-overhead-free
    differential. DRAM scratch is double-buffered across reps so rep r+1's
    phase A can overlap rep r's tail without WAR hazards."""
    nc = bacc.Bacc("TRN2", target_bir_lowering=False, debug=False, num_devices=W)

    a_in = nc.dram_tensor("a_c", [128, RL], MMD, kind="ExternalInput")
    x_in = nc.dram_tensor("x_c", [128, RL], MMD, kind="ExternalInput")
    t1r_in = nc.dram_tensor("t1r", [B, RL], F32, kind="ExternalInput")
    t1i_in = nc.dram_tensor("t1i", [B, RL], F32, kind="ExternalInput")
    dr_in = nc.dram_tensor("dr", [B, B], MMD, kind="ExternalInput")
    di_in = nc.dram_tensor("di", [B, B], MMD, kind="ExternalInput")
    ndi_in = nc.dram_tensor("ndi", [B, B], MMD, kind="ExternalInput")
    t2r_in = nc.dram_tensor("t2r", [B, B], F32, kind="ExternalInput")
    t2i_in = nc.dram_tensor("t2i", [B, B], F32, kind="ExternalInput")
    aw1_in = nc.dram_tensor("aw1", [B, 128], MMD, kind="ExternalInput")
    aw2_in = nc.dram_tensor("aw2", [B, 128], MMD, kind="ExternalInput")
    y_out = nc.dram_tensor("y_c", [128, RL], F32, kind="ExternalOutput")

    rg = [list(range(W))]

    with tile.TileContext(nc) as tc:
        with tc.tile_pool(name="dram", bufs=1, space="DRAM") as dram, \
             tc.tile_pool(name="consts", bufs=1) as consts:
            nbuf = 2 if reps > 1 else 1
            cc1_ins = [dram.tile([W, 32, 2, RL], MMD, name=f"cc1_in{i}", tag=f"cc1_in{i}")
                       for i in range(nbuf)]
            cc1_outs = [dram.tile([W, 32, 2, RL], MMD, name=f"cc1_out{i}", tag=f"cc1_out{i}")
                        for i in range(nbuf)]
            cc2_ins = [dram.tile([W, 32, 2, 32, B], F32, name=f"cc2_in{i}", tag=f"cc2_in{i}")
                       for i in range(nbuf)]
            cc2_outs = [dram.tile([W, 32, 2, 32, B], F32, name=f"cc2_out{i}", tag=f"cc2_out{i}")
                        for i in range(nbuf)]

            # ---- constant tables in SBUF ----
            # D row-halves (128, 256); col-slices give 128x128 blocks.
            dr_row, di_row, ndi_row, t2r_row, t2i_row = [], [], [], [], []
            for p in range(2):
                for lst, src, dt_ in ((dr_row, dr_in, MMD), (di_row, di_in, MMD), (ndi_row, ndi_in, MMD),
                                 (t2r_row, t2r_in, F32), (t2i_row, t2i_in, F32)):
                    t = consts.tile([128, B], dt_, name=f"c_{src.name}_{p}", tag=f"c_{src.name}_{p}")
                    nc.sync.dma_start(t[:], src[128 * p:128 * (p + 1), :])
                    lst.append(t)
            aw1_blk, aw2_blk = [], []
            for p in range(2):
                for lst, src in ((aw1_blk, aw1_in), (aw2_blk, aw2_in)):
                    t = consts.tile([128, 128], MMD, name=f"c_{src.name}_{p}", tag=f"c_{src.name}_{p}")
                    nc.sync.dma_start(t[:], src[128 * p:128 * (p + 1), :])
                    lst.append(t)

            def emit(rep, cc1_in, cc1_out, cc2_in, cc2_out):
                sfx = f"_{rep}"
                # ================= Phase A: stage A + T1 twiddle =================
                with tc.tile_pool(name="a_io" + sfx, bufs=1) as a_io, \
                     tc.tile_pool(name="a_t1" + sfx, bufs=8) as a_t1, \
                     tc.tile_pool(name="a_tmp" + sfx, bufs=16) as a_tmp, \
                     tc.tile_pool(name="a_out" + sfx, bufs=6) as a_outp, \
                     tc.tile_pool(name="a_ps" + sfx, bufs=4, space="PSUM") as a_ps:
                    a_full = a_io.tile([128, RL], MMD)
                    nc.sync.dma_start(a_full[:], a_in[:, :])
                    x_full = a_io.tile([128, RL], MMD)
                    nc.sync.dma_start(x_full[:], x_in[:, :])

                    for c in range(NCHUNK):
                        a_sl = a_full[:, c * CH:(c + 1) * CH]
                        x_sl = x_full[:, c * CH:(c + 1) * CH]
                        for h in range(2):
                            hs = slice(128 * h, 128 * (h + 1))
                            ps_r = a_ps.tile([128, CH], F32, tag="ps")
                            ps_i = a_ps.tile([128, CH], F32, tag="ps")
                            nc.tensor.matmul(ps_r[:], _mm(dr_row[0][:, hs]), _mm(a_sl),
                                             start=True, stop=False)
                            nc.tensor.matmul(ps_i[:], _mm(dr_row[0][:, hs]), _mm(x_sl),
                                             start=True, stop=False)
                            nc.tensor.matmul(ps_r[:], _mm(ndi_row[0][:, hs]), _mm(x_sl),
                                             start=False, stop=True)
                            nc.tensor.matmul(ps_i[:], _mm(di_row[0][:, hs]), _mm(a_sl),
                                             start=False, stop=True)

                            t1r_t = a_t1.tile([128, CH], F32, tag="t1")
                            nc.sync.dma_start(t1r_t[:], t1r_in[hs, c * CH:(c + 1) * CH])
                            t1i_t = a_t1.tile([128, CH], F32, tag="t1")
                            nc.sync.dma_start(t1i_t[:], t1i_in[hs, c * CH:(c + 1) * CH])

                            # Y' = (ps_r + i ps_i) * (t1r + i t1i), packed [Re | Im]
                            out_t = a_outp.tile([128, 2 * CH], MMD, tag="aout")
                            m1 = a_tmp.tile([128, CH], F32, tag="tmp")
                            m2 = a_tmp.tile([128, CH], F32, tag="tmp")
                            m3 = a_tmp.tile([128, CH], F32, tag="tmp")
                            m4 = a_tmp.tile([128, CH], F32, tag="tmp")
                            nc.vector.tensor_mul(m1[:], ps_r[:], t1r_t[:])
                            nc.vector.tensor_mul(m2[:], ps_i[:], t1i_t[:])
                            nc.vector.tensor_mul(m3[:], ps_r[:], t1i_t[:])
                            nc.vector.tensor_mul(m4[:], ps_i[:], t1r_t[:])
                            nc.gpsimd.tensor_sub(out_t[:, 0:CH], m1[:], m2[:])
                            nc.gpsimd.tensor_add(out_t[:, CH:2 * CH], m3[:], m4[:])

                            # store to A2A-1 input: dims (j=4, k1l=32, plane=2, rl=512)
                            nc.sync.dma_start(
                                cc1_in[4 * h:4 * (h + 1), :, :, c * CH:(c + 1) * CH],
                                out_t[:])

                nc.gpsimd.collective_compute(
                    "AllToAll", mybir.AluOpType.bypass, replica_groups=rg,
                    ins=[cc1_in.opt()], outs=[cc1_out.opt()])

                # ================= Middle: per-k1 row FFT + square ================
                with tc.tile_pool(name="m_in" + sfx, bufs=16) as m_in, \
                     tc.tile_pool(name="m_sb" + sfx, bufs=16) as m_sb, \
                     tc.tile_pool(name="m_out" + sfx, bufs=12) as m_out, \
                     tc.tile_pool(name="m_ps" + sfx, bufs=8, space="PSUM") as m_ps:
                    for k1l in range(32):
                        # load Y[k1] as (n2, n3), split in n2-halves, planes
                        y_t = []  # [n2h][plane]
                        for n2h in range(2):
                            row = []
                            for pl in range(2):
                                t = m_in.tile([128, B], MMD, tag="yin")
                                nc.sync.dma_start(
                                    t[:], cc1_out[4 * n2h:4 * (n2h + 1), k1l, pl, :])
                                row.append(t)
                            y_t.append(row)

                        # stage B (data as weights): Z^T (n3, k2) in PSUM
                        zt_ps = []  # [n3h][plane]
                        for n3h in range(2):
                            ns = slice(128 * n3h, 128 * (n3h + 1))
                            zr = m_ps.tile([128, B], F32, tag="mps")
                            zi = m_ps.tile([128, B], F32, tag="mps")
                            for n2h in range(2):
                                st = n2h == 0
                                sp = n2h == 1
                                nc.tensor.matmul(zr[:], _mm(y_t[n2h][0][:, ns]), _mm(dr_row[n2h][:]),
                                                 start=st, stop=False, skip_group_check=True)
                                nc.tensor.matmul(zi[:], _mm(y_t[n2h][0][:, ns]), _mm(di_row[n2h][:]),
                                                 start=st, stop=False, skip_group_check=True)
                                nc.tensor.matmul(zr[:], _mm(y_t[n2h][1][:, ns]), _mm(ndi_row[n2h][:]),
                                                 start=False, stop=sp, skip_group_check=True)
                                nc.tensor.matmul(zi[:], _mm(y_t[n2h][1][:, ns]), _mm(dr_row[n2h][:]),
                                                 start=False, stop=sp, skip_group_check=True)
                            zt_ps.append((zr, zi))

                        # T2 twiddle (Z^T layout: mult by T2^T rows) -> SBUF
                        zt_sb = []
                        for n3h in range(2):
                            zr, zi = zt_ps[n3h]
                            or_ = m_sb.tile([128, B], MMD, tag="zt")
                            oi_ = m_sb.tile([128, B], MMD, tag="zt")
                            p1 = m_sb.tile([128, B], F32, tag="mtmp")
                            p2 = m_sb.tile([128, B], F32, tag="mtmp")
                            p3 = m_sb.tile([128, B], F32, tag="mtmp")
                            p4 = m_sb.tile([128, B], F32, tag="mtmp")
                            nc.vector.tensor_mul(p1[:], zr[:], t2r_row[n3h][:])
                            nc.vector.tensor_mul(p2[:], zi[:], t2i_row[n3h][:])
                            nc.vector.tensor_mul(p3[:], zr[:], t2i_row[n3h][:])
                            nc.vector.tensor_mul(p4[:], zi[:], t2r_row[n3h][:])
                            nc.gpsimd.tensor_sub(or_[:], p1[:], p2[:])
                            nc.gpsimd.tensor_add(oi_[:], p3[:], p4[:])
                            zt_sb.append((or_, oi_))

                        # stage C (DFT stationary): U^T (k3, k2) in PSUM
                        ut_ps = []
                        for k3h in range(2):
                            ks = slice(128 * k3h, 128 * (k3h + 1))
                            ur = m_ps.tile([128, B], F32, tag="mps")
                            ui = m_ps.tile([128, B], F32, tag="mps")
                            for n3h in range(2):
                                st = n3h == 0
                                sp = n3h == 1
                                nc.tensor.matmul(ur[:], _mm(dr_row[n3h][:, ks]), _mm(zt_sb[n3h][0][:]),
                                                 start=st, stop=False, skip_group_check=True)
                                nc.tensor.matmul(ui[:], _mm(di_row[n3h][:, ks]), _mm(zt_sb[n3h][0][:]),
                                                 start=st, stop=False, skip_group_check=True)
                                nc.tensor.matmul(ur[:], _mm(ndi_row[n3h][:, ks]), _mm(zt_sb[n3h][1][:]),
                                                 start=False, stop=sp, skip_group_check=True)
                                nc.tensor.matmul(ui[:], _mm(dr_row[n3h][:, ks]), _mm(zt_sb[n3h][1][:]),
                                                 start=False, stop=sp, skip_group_check=True)
                            ut_ps.append((ur, ui))

                        # square: S = U^2 (k3, k2) -> SBUF
                        s_sb = []
                        for k3h in range(2):
                            ur, ui = ut_ps[k3h]
                            sr = m_sb.tile([128, B], MMD, tag="ssb")
                            si = m_sb.tile([128, B], MMD, tag="ssb")
                            uc = m_sb.tile([128, B], F32, tag="mtmp")
                            q1 = m_sb.tile([128, B], F32, tag="mtmp")
                            q2 = m_sb.tile([128, B], F32, tag="mtmp")
                            # only one PSUM operand allowed per vector op: stage ur in SBUF
                            nc.scalar.copy(uc[:], ur[:])
                            nc.vector.tensor_add(q1[:], uc[:], ui[:])
                            nc.vector.tensor_sub(q2[:], uc[:], ui[:])
                            nc.vector.scalar_tensor_tensor(
                                si[:], uc[:], 2.0, ui[:],
                                mybir.AluOpType.mult, mybir.AluOpType.mult)
                            nc.gpsimd.tensor_mul(sr[:], q1[:], q2[:])
                            s_sb.append((sr, si))

                        # stage C' (data as weights): Z2 (k2, n3) in PSUM
                        z2_ps = []
                        for k2h in range(2):
                            ks = slice(128 * k2h, 128 * (k2h + 1))
                            zr = m_ps.tile([128, B], F32, tag="mps")
                            zi = m_ps.tile([128, B], F32, tag="mps")
                            for k3h in range(2):
                                st = k3h == 0
                                sp = k3h == 1
                                nc.tensor.matmul(zr[:], _mm(s_sb[k3h][0][:, ks]), _mm(dr_row[k3h][:]),
                                                 start=st, stop=False, skip_group_check=True)
                                nc.tensor.matmul(zi[:], _mm(s_sb[k3h][0][:, ks]), _mm(ndi_row[k3h][:]),
                                                 start=st, stop=False, skip_group_check=True)
                                nc.tensor.matmul(zr[:], _mm(s_sb[k3h][1][:, ks]), _mm(di_row[k3h][:]),
                                                 start=False, stop=sp, skip_group_check=True)
                                nc.tensor.matmul(zi[:], _mm(s_sb[k3h][1][:, ks]), _mm(dr_row[k3h][:]),
                                                 start=False, stop=sp, skip_group_check=True)
                            z2_ps.append((zr, zi))

                        # conj(T2) twiddle (natural (k2, n3) layout) -> SBUF
                        y2_sb = []
                        for k2h in range(2):
                            zr, zi = z2_ps[k2h]
                            or_ = m_sb.tile([128, B], MMD, tag="y2")
                            oi_ = m_sb.tile([128, B], MMD, tag="y2")
                            p1 = m_sb.tile([128, B], F32, tag="mtmp")
                            p2 = m_sb.tile([128, B], F32, tag="mtmp")
                            p3 = m_sb.tile([128, B], F32, tag="mtmp")
                            p4 = m_sb.tile([128, B], F32, tag="mtmp")
                            nc.vector.tensor_mul(p1[:], zr[:], t2r_row[k2h][:])
                            nc.vector.tensor_mul(p2[:], zi[:], t2i_row[k2h][:])
                            nc.vector.tensor_mul(p3[:], zi[:], t2r_row[k2h][:])
                            nc.vector.tensor_mul(p4[:], zr[:], t2i_row[k2h][:])
                            nc.gpsimd.tensor_add(or_[:], p1[:], p2[:])
                            nc.gpsimd.tensor_sub(oi_[:], p3[:], p4[:])
                            y2_sb.append((or_, oi_))

                        # stage B' (DFT stationary, conj D): Y' (n2, n3) in PSUM
                        for n2h in range(2):
                            ns = slice(128 * n2h, 128 * (n2h + 1))
                            yr = m_ps.tile([128, B], F32, tag="mps")
                            yi = m_ps.tile([128, B], F32, tag="mps")
                            for k2h in range(2):
                                st = k2h == 0
                                sp = k2h == 1
                                nc.tensor.matmul(yr[:], _mm(dr_row[k2h][:, ns]), _mm(y2_sb[k2h][0][:]),
                                                 start=st, stop=False, skip_group_check=True)
                                nc.tensor.matmul(yi[:], _mm(dr_row[k2h][:, ns]), _mm(y2_sb[k2h][1][:]),
                                                 start=st, stop=False, skip_group_check=True)
                                nc.tensor.matmul(yr[:], _mm(di_row[k2h][:, ns]), _mm(y2_sb[k2h][1][:]),
                                                 start=False, stop=sp, skip_group_check=True)
                                nc.tensor.matmul(yi[:], _mm(ndi_row[k2h][:, ns]), _mm(y2_sb[k2h][0][:]),
                                                 start=False, stop=sp, skip_group_check=True)
                            # copy to SBUF and store to A2A-2 input
                            for pl, ps in ((0, yr), (1, yi)):
                                o = m_out.tile([128, B], F32, tag="mout")
                                nc.scalar.copy(o[:], ps[:])
                                nc.sync.dma_start(
                                    cc2_in[4 * n2h:4 * (n2h + 1), k1l, pl, :, :], o[:])

                nc.gpsimd.collective_compute(
                    "AllToAll", mybir.AluOpType.bypass, replica_groups=rg,
                    ins=[cc2_in.opt()], outs=[cc2_out.opt()])

                # ============ Phase A': conj(T1), inverse stage A (Im only) ============
                with tc.tile_pool(name="f_in" + sfx, bufs=16) as f_in, \
                     tc.tile_pool(name="f_t1" + sfx, bufs=8) as f_t1, \
                     tc.tile_pool(name="f_tmp" + sfx, bufs=16) as f_tmp, \
                     tc.tile_pool(name="f_out" + sfx, bufs=6) as f_outp, \
                     tc.tile_pool(name="f_ps" + sfx, bufs=4, space="PSUM") as f_ps:
                    for c in range(NCHUNK):
                        ps_o = f_ps.tile([128, CH], F32, tag="fps")
                        for h in range(2):
                            hs = slice(128 * h, 128 * (h + 1))
                            pr = f_in.tile([128, CH], F32, tag="pin")
                            nc.sync.dma_start(
                                pr[:], cc2_out[4 * h:4 * (h + 1), :, 0, 2 * c:2 * (c + 1), :])
                            pi = f_in.tile([128, CH], F32, tag="pin")
                            nc.sync.dma_start(
                                pi[:], cc2_out[4 * h:4 * (h + 1), :, 1, 2 * c:2 * (c + 1), :])
                            t1r_t = f_t1.tile([128, CH], F32, tag="ft1")
                            nc.sync.dma_start(t1r_t[:], t1r_in[hs, c * CH:(c + 1) * CH])
                            t1i_t = f_t1.tile([128, CH], F32, tag="ft1")
                            nc.sync.dma_start(t1i_t[:], t1i_in[hs, c * CH:(c + 1) * CH])

                            # Yf = P * conj(T1)
                            yfr = f_tmp.tile([128, CH], MMD, tag="yf")
                            yfi = f_tmp.tile([128, CH], MMD, tag="yf")
                            p1 = f_tmp.tile([128, CH], F32, tag="ftmp")
                            p2 = f_tmp.tile([128, CH], F32, tag="ftmp")
                            p3 = f_tmp.tile([128, CH], F32, tag="ftmp")
                            p4 = f_tmp.tile([128, CH], F32, tag="ftmp")
                            nc.vector.tensor_mul(p1[:], pr[:], t1r_t[:])
                            nc.gpsimd.tensor_mul(p2[:], pi[:], t1i_t[:])
                            nc.vector.tensor_mul(p3[:], pi[:], t1r_t[:])
                            nc.gpsimd.tensor_mul(p4[:], pr[:], t1i_t[:])
                            nc.vector.tensor_add(yfr[:], p1[:], p2[:])
                            nc.vector.tensor_sub(yfi[:], p3[:], p4[:])

                            st = h == 0
                            sp = h == 1
                            nc.tensor.matmul(ps_o[:], _mm(aw1_blk[h][:]), _mm(yfi[:]),
                                             start=st, stop=False, skip_group_check=True)
                            nc.tensor.matmul(ps_o[:], _mm(aw2_blk[h][:]), _mm(yfr[:]),
                                             start=False, stop=sp, skip_group_check=True)

                        o = f_outp.tile([128, CH], F32, tag="fout")
                        nc.scalar.copy(o[:], ps_o[:])
                        nc.sync.dma_start(y_out[:, c * CH:(c + 1) * CH], o[:])

            for rep in range(reps):
                bi = rep % nbuf
                emit(rep, cc1_ins[bi], cc1_outs[bi], cc2_ins[bi], cc2_outs[bi])

    nc.compile()
    return nc


_NC = None
_TABLES = None


def _tables():
    global _TABLES
    if _TABLES is None:
        k = np.arange(B)
        D = np.exp(-2j * np.pi * np.outer(k, k) / B)
        T2 = np.exp(-2j * np.pi * np.outer(k, k) / R)
        s = 1.0 / (2.0 * M)
        dr = np.ascontiguousarray(D.real.astype(np.float32))
        di = np.ascontiguousarray(D.imag.astype(np.float32))
        t1s = []
        for c in range(W):
            r = np.arange(c * RL, (c + 1) * RL)
            T1 = np.exp(-2j * np.pi * np.outer(k, r) / M)
            t1s.append((np.ascontiguousarray(T1.real.astype(np.float32)),
                        np.ascontiguousarray(T1.imag.astype(np.float32))))
        _TABLES = dict(
            dr=dr, di=di, ndi=np.ascontiguousarray(-di),
            t2r=np.ascontiguousarray(T2.real.astype(np.float32)),
            t2i=np.ascontiguousarray(T2.imag.astype(np.float32)),
            aw1=np.ascontiguousarray((s * D.real[:, :128]).astype(np.float32)),
            aw2=np.ascontiguousarray((-s * D.imag[:, :128]).astype(np.float32)),
            t1s=t1s,
        )
    return _TABLES


def _in_map(a_c, x_c, tb, c):
    return dict(
        a_c=np.ascontiguousarray(a_c),
        x_c=np.ascontiguousarray(x_c),
        t1r=tb["t1s"][c][0], t1i=tb["t1s"][c][1],
        dr=tb["dr"], di=tb["di"], ndi=tb["ndi"],
        t2r=tb["t2r"], t2i=tb["t2i"],
        aw1=tb["aw1"], aw2=tb["aw2"],
    )


def kernel(a, x, _want_trace=False, **_unused):
    global _NC
    a = np.asarray(a, dtype=np.float32)
    x = np.asarray(x, dtype=np.float32)
    tb = _tables()
    if _NC is None:
        _NC = build_nc()

    a3 = a.reshape(128, W, RL)
    x3 = x.reshape(128, W, RL)
    in_maps = []
    for c in range(W):
        in_maps.append(_in_map(a3[:, c, :], x3[:, c, :], tb, c))
    res = run_bass_kernel_spmd(_NC, in_maps, core_ids=list(range(W)),
                               trace=_want_trace)
    full = np.empty((128, R), dtype=np.float32)
    for c in range(W):
        full[:, c * RL:(c + 1) * RL] = res.results[c]["y_c"]
    out = full.reshape(-1)
    if _want_trace:
        return out, res
    return out


# revision 13
# speedup vs baseline: 64.4649x; 1.1634x over previous
"""FFT-based linear convolution of two 2^23-point real signals on 8 trn2 NeuronCores.

Math: conv(a, x) = Im(ifft(fft(a + i*x)^2)) / 2, with the 2^24-point FFT done as a
3-factor (256^3) matmul FFT. Stage A (over n1) is computed r-sharded across cores,
one AllToAll reshards to k1-sharded for the middle row-FFTs (stages B, C), the
pointwise square happens in the digit-reversed domain, then the inverse stages
(C', B') run locally, a second AllToAll reshards back, and inverse stage A'
produces only the imaginary part of the first half of the time-domain signal.

The 4-step twiddle T1[k1, r] = exp(-2πi k1 r / M) factors over r = 256*n2 + n3
as u_{k1}[n2] * w_{k1}[n3]. Instead of streaming a [256, 8192] T1 table from HBM
(16 MB/core/phase), u is applied in the middle phase as a per-partition complex
scale on the (n2, n3) tiles and w as a per-partition scale on the (n3, k2) tiles
after stage B; the inverse applies conj(u) (with the 1/2M normalization folded
in) after stage B', and phase A' applies conj(w)[k, n3] from a small SBUF-
resident [256, 256] table. Inter-phase payloads (both AllToAlls) are fp16.
"""
import os
import numpy as np

os.environ.setdefault("JAX_PLATFORMS", "")
import jax

jax.config.update("jax_compilation_cache_dir", "/tmp/jax_neff_cache")
jax.config.update("jax_persistent_cache_min_entry_size_bytes", -1)
jax.config.update("jax_persistent_cache_min_compile_time_secs", 0)

import concourse.bass as bass
import concourse.tile as tile
from concourse import bacc, mybir
from concourse.bass_utils import run_bass_kernel_spmd

N = 8388608          # input length
M = 2 * N            # FFT size = 2^24
B = 256              # radix
R = B * B            # 65536
W = 8                # cores
RL = R // W          # 8192 columns of r per core
CH = 512             # free-dim chunk in stages A / A'
NCHUNK = RL // CH    # 16
F32 = mybir.dt.float32
F16 = mybir.dt.float16
AOP = mybir.AluOpType

# matmul operands use float32r (full-rate PE, ~13-bit mantissa). Operands must be
# produced by a rounding op: gpsimd cast-DMA for host data, f32r-output compute
# ops for device intermediates. Set USE_F32R=False to fall back to exact fp32.
USE_F32R = True
MMD = mybir.dt.float32r if USE_F32R else F32
PAY = F16            # inter-phase (AllToAll) payload dtype


def _mm(ap):
    return ap


def build_nc(reps=1, phases=5):
    """Build the kernel NEFF. reps>1 runs the full pipeline that many times
    back-to-back inside one NEFF (same inputs, same output) — used by test.py
    to measure per-execution device time as a dispatch-overhead-free
    differential. DRAM scratch is double-buffered across reps so rep r+1's
    phase A can overlap rep r's tail without WAR hazards.

    phases<5 truncates the pipeline (1=A, 2=+A2A1, 3=+middle, 4=+A2A2,
    5=full) — timing-only ablation builds; their output is garbage."""
    nc = bacc.Bacc("TRN2", target_bir_lowering=False, debug=False, num_devices=W)

    a_in = nc.dram_tensor("a_c", [128, RL], MMD, kind="ExternalInput")
    x_in = nc.dram_tensor("x_c", [128, RL], MMD, kind="ExternalInput")
    dr_in = nc.dram_tensor("dr", [B, B], MMD, kind="ExternalInput")
    di_in = nc.dram_tensor("di", [B, B], MMD, kind="ExternalInput")
    ndi_in = nc.dram_tensor("ndi", [B, B], MMD, kind="ExternalInput")
    t2r_in = nc.dram_tensor("t2r", [B, B], F32, kind="ExternalInput")
    t2i_in = nc.dram_tensor("t2i", [B, B], F32, kind="ExternalInput")
    aw1_in = nc.dram_tensor("aw1", [B, 128], MMD, kind="ExternalInput")
    aw2_in = nc.dram_tensor("aw2", [B, 128], MMD, kind="ExternalInput")
    # forward/inverse 4-step twiddle factors (see module docstring):
    #   ur/ui [p, n2h*32+k1l] = cos/sin(2π k1 n2 / 65536), k1 = 32*core + k1l
    #   sur/sui = same scaled by 1/(2M);  wr/wi = cos/sin(2π k1 n3 / M)
    #   cwr/cwi [k, n3] = cos/-sin(2π k n3 / M)  (phase A', core-independent)
    ur_in = nc.dram_tensor("ur", [128, 64], F32, kind="ExternalInput")
    ui_in = nc.dram_tensor("ui", [128, 64], F32, kind="ExternalInput")
    sur_in = nc.dram_tensor("sur", [128, 64], F32, kind="ExternalInput")
    sui_in = nc.dram_tensor("sui", [128, 64], F32, kind="ExternalInput")
    wr_in = nc.dram_tensor("wr", [128, 64], F32, kind="ExternalInput")
    wi_in = nc.dram_tensor("wi", [128, 64], F32, kind="ExternalInput")
    cwr_in = nc.dram_tensor("cwr", [B, B], PAY, kind="ExternalInput")
    cwi_in = nc.dram_tensor("cwi", [B, B], PAY, kind="ExternalInput")
    y_out = nc.dram_tensor("y_c", [128, RL], F32, kind="ExternalOutput")

    rg = [list(range(W))]

    with tile.TileContext(nc) as tc:
        with tc.tile_pool(name="dram", bufs=1, space="DRAM") as dram, \
             tc.tile_pool(name="consts", bufs=1) as consts:
            nbuf = 2 if reps > 1 else 1
            cc1_ins = [dram.tile([W, 32, 2, RL], PAY, name=f"cc1_in{i}", tag=f"cc1_in{i}")
                       for i in range(nbuf)]
            cc1_outs = [dram.tile([W, 32, 2, RL], PAY, name=f"cc1_out{i}", tag=f"cc1_out{i}")
                        for i in range(nbuf)]
            cc2_ins = [dram.tile([W, 32, 2, 32, B], PAY, name=f"cc2_in{i}", tag=f"cc2_in{i}")
                       for i in range(nbuf)]
            cc2_outs = [dram.tile([W, 32, 2, 32, B], PAY, name=f"cc2_out{i}", tag=f"cc2_out{i}")
                        for i in range(nbuf)]

            # ---- constant tables in SBUF ----
            # D row-halves (128, 256); col-slices give 128x128 blocks.
            dr_row, di_row, ndi_row, t2r_row, t2i_row = [], [], [], [], []
            for p in range(2):
                for lst, src, dt_ in ((dr_row, dr_in, MMD), (di_row, di_in, MMD), (ndi_row, ndi_in, MMD),
                                 (t2r_row, t2r_in, F32), (t2i_row, t2i_in, F32)):
                    t = consts.tile([128, B], dt_, name=f"c_{src.name}_{p}", tag=f"c_{src.name}_{p}")
                    nc.sync.dma_start(t[:], src[128 * p:128 * (p + 1), :])
                    lst.append(t)
            aw1_blk, aw2_blk = [], []
            for p in range(2):
                for lst, src in ((aw1_blk, aw1_in), (aw2_blk, aw2_in)):
                    t = consts.tile([128, 128], MMD, name=f"c_{src.name}_{p}", tag=f"c_{src.name}_{p}")
                    nc.sync.dma_start(t[:], src[128 * p:128 * (p + 1), :])
                    lst.append(t)
            uw = {}
            for src in (ur_in, ui_in, sur_in, sui_in, wr_in, wi_in):
                t = consts.tile([128, 64], F32, name=f"c_{src.name}", tag=f"c_{src.name}")
                nc.sync.dma_start(t[:], src[:, :])
                uw[src.name] = t
            cwr_h, cwi_h = [], []
            for p in range(2):
                for lst, src in ((cwr_h, cwr_in), (cwi_h, cwi_in)):
                    t = consts.tile([128, B], PAY, name=f"c_{src.name}_{p}", tag=f"c_{src.name}_{p}")
                    nc.sync.dma_start(t[:], src[128 * p:128 * (p + 1), :])
                    lst.append(t)

            def emit(rep, cc1_in, cc1_out, cc2_in, cc2_out):
                sfx = f"_{rep}"
                # ================= Phase A: stage A (no twiddle) =================
                with tc.tile_pool(name="a_io" + sfx, bufs=1) as a_io, \
                     tc.tile_pool(name="a_out" + sfx, bufs=6) as a_outp, \
                     tc.tile_pool(name="a_ps" + sfx, bufs=4, space="PSUM") as a_ps:
                    a_full = a_io.tile([128, RL], MMD)
                    nc.sync.dma_start(a_full[:], a_in[:, :])
                    x_full = a_io.tile([128, RL], MMD)
                    nc.sync.dma_start(x_full[:], x_in[:, :])

                    for c in range(NCHUNK):
                        a_sl = a_full[:, c * CH:(c + 1) * CH]
                        x_sl = x_full[:, c * CH:(c + 1) * CH]
                        for h in range(2):
                            hs = slice(128 * h, 128 * (h + 1))
                            ps_r = a_ps.tile([128, CH], F32, tag="ps")
                            ps_i = a_ps.tile([128, CH], F32, tag="ps")
                            nc.tensor.matmul(ps_r[:], _mm(dr_row[0][:, hs]), _mm(a_sl),
                                             start=True, stop=False)
                            nc.tensor.matmul(ps_i[:], _mm(dr_row[0][:, hs]), _mm(x_sl),
                                             start=True, stop=False)
                            nc.tensor.matmul(ps_r[:], _mm(ndi_row[0][:, hs]), _mm(x_sl),
                                             start=False, stop=True)
                            nc.tensor.matmul(ps_i[:], _mm(di_row[0][:, hs]), _mm(a_sl),
                                             start=False, stop=True)

                            # pack [Re | Im] as fp16 payload
                            out_t = a_outp.tile([128, 2 * CH], PAY, tag="aout")
                            nc.scalar.copy(out_t[:, 0:CH], ps_r[:])
                            nc.vector.tensor_scalar_add(out_t[:, CH:2 * CH], ps_i[:], 0.0)

                            # store to A2A-1 input: dims (j=4, k1l=32, plane=2, rl=512)
                            nc.sync.dma_start(
                                cc1_in[4 * h:4 * (h + 1), :, :, c * CH:(c + 1) * CH],
                                out_t[:])

                if phases < 2:
                    return
                nc.gpsimd.collective_compute(
                    "AllToAll", mybir.AluOpType.bypass, replica_groups=rg,
                    ins=[cc1_in.opt()], outs=[cc1_out.opt()])
                if phases < 3:
                    return

                # ================= Middle: per-k1 row FFT + square ================
                with tc.tile_pool(name="m_in" + sfx, bufs=16) as m_in, \
                     tc.tile_pool(name="m_sb" + sfx, bufs=16) as m_sb, \
                     tc.tile_pool(name="m_out" + sfx, bufs=12) as m_out, \
                     tc.tile_pool(name="m_ps" + sfx, bufs=8, space="PSUM") as m_ps:
                    for k1l in range(32):
                        # load Y[k1] as (n2, n3), split in n2-halves, planes
                        y_t = []  # [n2h][plane]
                        for n2h in range(2):
                            row = []
                            for pl in range(2):
                                t = m_in.tile([128, B], PAY, tag="yin")
                                nc.sync.dma_start(
                                    t[:], cc1_out[4 * n2h:4 * (n2h + 1), k1l, pl, :])
                                row.append(t)
                            y_t.append(row)

                        # u-twiddle: per-partition complex scale by exp(-iθ),
                        # θ = 2π k1 n2 / 65536  →  yu (n2, n3) in f32r
                        yu = []  # [n2h] -> (re, im)
                        for n2h in range(2):
                            j = n2h * 32 + k1l
                            yr, yi = y_t[n2h]
                            ucos = uw["ur"][:, j:j + 1]
                            usin = uw["ui"][:, j:j + 1]
                            ta = m_sb.tile([128, B], PAY, tag="mtmp")
                            nc.scalar.mul(ta[:], yi[:], usin)
                            tb = m_sb.tile([128, B], PAY, tag="mtmp")
                            nc.scalar.mul(tb[:], yr[:], usin)
                            pr_ = m_sb.tile([128, B], MMD, tag="yu")
                            pi_ = m_sb.tile([128, B], MMD, tag="yu")
                            nc.vector.scalar_tensor_tensor(
                                pr_[:], yr[:], ucos, ta[:], AOP.mult, AOP.add)
                            nc.vector.scalar_tensor_tensor(
                                pi_[:], yi[:], ucos, tb[:], AOP.mult, AOP.subtract)
                            yu.append((pr_, pi_))

                        # stage B (data as weights): Z^T (n3, k2) in PSUM
                        zt_ps = []  # [n3h][plane]
                        for n3h in range(2):
                            ns = slice(128 * n3h, 128 * (n3h + 1))
                            zr = m_ps.tile([128, B], F32, tag="mps")
                            zi = m_ps.tile([128, B], F32, tag="mps")
                            for n2h in range(2):
                                st = n2h == 0
                                sp = n2h == 1
                                nc.tensor.matmul(zr[:], _mm(yu[n2h][0][:, ns]), _mm(dr_row[n2h][:]),
                                                 start=st, stop=False, skip_group_check=True)
                                nc.tensor.matmul(zi[:], _mm(yu[n2h][0][:, ns]), _mm(di_row[n2h][:]),
                                                 start=st, stop=False, skip_group_check=True)
                                nc.tensor.matmul(zr[:], _mm(yu[n2h][1][:, ns]), _mm(ndi_row[n2h][:]),
                                                 start=False, stop=sp, skip_group_check=True)
                                nc.tensor.matmul(zi[:], _mm(yu[n2h][1][:, ns]), _mm(dr_row[n2h][:]),
                                                 start=False, stop=sp, skip_group_check=True)
                            zt_ps.append((zr, zi))

                        # w-twiddle (per-partition, φ = 2π k1 n3 / M) then T2
                        # twiddle (Z^T layout: mult by T2^T rows) -> SBUF f32r
                        zt_sb = []
                        for n3h in range(2):
                            jw = n3h * 32 + k1l
                            zr, zi = zt_ps[n3h]
                            wcos = uw["wr"][:, jw:jw + 1]
                            wsin = uw["wi"][:, jw:jw + 1]
                            ta = m_sb.tile([128, B], F32, tag="mtmp")
                            nc.scalar.mul(ta[:], zi[:], wsin)
                            tb = m_sb.tile([128, B], F32, tag="mtmp")
                            nc.scalar.mul(tb[:], zr[:], wsin)
                            mr = m_sb.tile([128, B], F32, tag="mtmp")
                            mi = m_sb.tile([128, B], F32, tag="mtmp")
                            nc.vector.scalar_tensor_tensor(
                                mr[:], zr[:], wcos, ta[:], AOP.mult, AOP.add)
                            nc.vector.scalar_tensor_tensor(
                                mi[:], zi[:], wcos, tb[:], AOP.mult, AOP.subtract)

                            or_ = m_sb.tile([128, B], MMD, tag="zt")
                            oi_ = m_sb.tile([128, B], MMD, tag="zt")
                            p1 = m_sb.tile([128, B], F32, tag="mtmp")
                            p2 = m_sb.tile([128, B], F32, tag="mtmp")
                            p3 = m_sb.tile([128, B], F32, tag="mtmp")
                            p4 = m_sb.tile([128, B], F32, tag="mtmp")
                            nc.vector.tensor_mul(p1[:], mr[:], t2r_row[n3h][:])
                            nc.gpsimd.tensor_mul(p2[:], mi[:], t2i_row[n3h][:])
                            nc.gpsimd.tensor_mul(p3[:], mi[:], t2r_row[n3h][:])
                            nc.vector.tensor_mul(p4[:], mr[:], t2i_row[n3h][:])
                            nc.gpsimd.tensor_sub(or_[:], p1[:], p2[:])
                            nc.vector.tensor_add(oi_[:], p3[:], p4[:])
                            zt_sb.append((or_, oi_))

                        # stage C (DFT stationary): U^T (k3, k2) in PSUM
                        ut_ps = []
                        for k3h in range(2):
                            ks = slice(128 * k3h, 128 * (k3h + 1))
                            ur = m_ps.tile([128, B], F32, tag="mps")
                            ui = m_ps.tile([128, B], F32, tag="mps")
                            for n3h in range(2):
                                st = n3h == 0
                                sp = n3h == 1
                                nc.tensor.matmul(ur[:], _mm(dr_row[n3h][:, ks]), _mm(zt_sb[n3h][0][:]),
                                                 start=st, stop=False, skip_group_check=True)
                                nc.tensor.matmul(ui[:], _mm(di_row[n3h][:, ks]), _mm(zt_sb[n3h][0][:]),
                                                 start=st, stop=False, skip_group_check=True)
                                nc.tensor.matmul(ur[:], _mm(ndi_row[n3h][:, ks]), _mm(zt_sb[n3h][1][:]),
                                                 start=False, stop=sp, skip_group_check=True)
                                nc.tensor.matmul(ui[:], _mm(dr_row[n3h][:, ks]), _mm(zt_sb[n3h][1][:]),
                                                 start=False, stop=sp, skip_group_check=True)
                            ut_ps.append((ur, ui))

                        # square: S = U^2 (k3, k2) -> SBUF
                        s_sb = []
                        for k3h in range(2):
                            ur, ui = ut_ps[k3h]
                            sr = m_sb.tile([128, B], MMD, tag="ssb")
                            si = m_sb.tile([128, B], MMD, tag="ssb")
                            uc = m_sb.tile([128, B], F32, tag="mtmp")
                            q1 = m_sb.tile([128, B], F32, tag="mtmp")
                            q2 = m_sb.tile([128, B], F32, tag="mtmp")
                            # only one PSUM operand allowed per vector op: stage ur in SBUF
                            nc.scalar.copy(uc[:], ur[:])
                            nc.vector.tensor_add(q1[:], uc[:], ui[:])
                            nc.vector.tensor_sub(q2[:], uc[:], ui[:])
                            nc.vector.scalar_tensor_tensor(
                                si[:], uc[:], 2.0, ui[:],
                                mybir.AluOpType.mult, mybir.AluOpType.mult)
                            nc.gpsimd.tensor_mul(sr[:], q1[:], q2[:])
                            s_sb.append((sr, si))

                        # stage C' (data as weights): Z2 (k2, n3) in PSUM
                        z2_ps = []
                        for k2h in range(2):
                            ks = slice(128 * k2h, 128 * (k2h + 1))
                            zr = m_ps.tile([128, B], F32, tag="mps")
                            zi = m_ps.tile([128, B], F32, tag="mps")
                            for k3h in range(2):
                                st = k3h == 0
                                sp = k3h == 1
                                nc.tensor.matmul(zr[:], _mm(s_sb[k3h][0][:, ks]), _mm(dr_row[k3h][:]),
                                                 start=st, stop=False, skip_group_check=True)
                                nc.tensor.matmul(zi[:], _mm(s_sb[k3h][0][:, ks]), _mm(ndi_row[k3h][:]),
                                                 start=st, stop=False, skip_group_check=True)
                                nc.tensor.matmul(zr[:], _mm(s_sb[k3h][1][:, ks]), _mm(di_row[k3h][:]),
                                                 start=False, stop=sp, skip_group_check=True)
                                nc.tensor.matmul(zi[:], _mm(s_sb[k3h][1][:, ks]), _mm(dr_row[k3h][:]),
                                                 start=False, stop=sp, skip_group_check=True)
                            z2_ps.append((zr, zi))

                        # conj(T2) twiddle (natural (k2, n3) layout) -> SBUF
                        y2_sb = []
                        for k2h in range(2):
                            zr, zi = z2_ps[k2h]
                            or_ = m_sb.tile([128, B], MMD, tag="y2")
                            oi_ = m_sb.tile([128, B], MMD, tag="y2")
                            p1 = m_sb.tile([128, B], F32, tag="mtmp")
                            p2 = m_sb.tile([128, B], F32, tag="mtmp")
                            p3 = m_sb.tile([128, B], F32, tag="mtmp")
                            p4 = m_sb.tile([128, B], F32, tag="mtmp")
                            nc.vector.tensor_mul(p1[:], zr[:], t2r_row[k2h][:])
                            nc.vector.tensor_mul(p2[:], zi[:], t2i_row[k2h][:])
                            nc.vector.tensor_mul(p3[:], zi[:], t2r_row[k2h][:])
                            nc.vector.tensor_mul(p4[:], zr[:], t2i_row[k2h][:])
                            nc.gpsimd.tensor_add(or_[:], p1[:], p2[:])
                            nc.gpsimd.tensor_sub(oi_[:], p3[:], p4[:])
                            y2_sb.append((or_, oi_))

                        # stage B' (DFT stationary, conj D): Y' (n2, n3) in PSUM,
                        # then conj(u)-twiddle (1/2M folded in) -> fp16 payload
                        for n2h in range(2):
                            ns = slice(128 * n2h, 128 * (n2h + 1))
                            yr = m_ps.tile([128, B], F32, tag="mps")
                            yi = m_ps.tile([128, B], F32, tag="mps")
                            for k2h in range(2):
                                st = k2h == 0
                                sp = k2h == 1
                                nc.tensor.matmul(yr[:], _mm(dr_row[k2h][:, ns]), _mm(y2_sb[k2h][0][:]),
                                                 start=st, stop=False, skip_group_check=True)
                                nc.tensor.matmul(yi[:], _mm(dr_row[k2h][:, ns]), _mm(y2_sb[k2h][1][:]),
                                                 start=st, stop=False, skip_group_check=True)
                                nc.tensor.matmul(yr[:], _mm(di_row[k2h][:, ns]), _mm(y2_sb[k2h][1][:]),
                                                 start=False, stop=sp, skip_group_check=True)
                                nc.tensor.matmul(yi[:], _mm(ndi_row[k2h][:, ns]), _mm(y2_sb[k2h][0][:]),
                                                 start=False, stop=sp, skip_group_check=True)
                            j = n2h * 32 + k1l
                            scos = uw["sur"][:, j:j + 1]
                            ssin = uw["sui"][:, j:j + 1]
                            ta = m_out.tile([128, B], F32, tag="motmp")
                            nc.scalar.mul(ta[:], yi[:], ssin)
                            tb = m_out.tile([128, B], F32, tag="motmp")
                            nc.scalar.mul(tb[:], yr[:], ssin)
                            o_r = m_out.tile([128, B], PAY, tag="mout")
                            o_i = m_out.tile([128, B], PAY, tag="mout")
                            nc.vector.scalar_tensor_tensor(
                                o_r[:], yr[:], scos, ta[:], AOP.mult, AOP.subtract)
                            nc.vector.scalar_tensor_tensor(
                                o_i[:], yi[:], scos, tb[:], AOP.mult, AOP.add)
                            nc.sync.dma_start(
                                cc2_in[4 * n2h:4 * (n2h + 1), k1l, 0, :, :], o_r[:])
                            nc.sync.dma_start(
                                cc2_in[4 * n2h:4 * (n2h + 1), k1l, 1, :, :], o_i[:])

                if phases < 4:
                    return
                nc.gpsimd.collective_compute(
                    "AllToAll", mybir.AluOpType.bypass, replica_groups=rg,
                    ins=[cc2_in.opt()], outs=[cc2_out.opt()])
                if phases < 5:
                    return

                # ============ Phase A': conj(w), inverse stage A (Im only) ============
                with tc.tile_pool(name="f_in" + sfx, bufs=16) as f_in, \
                     tc.tile_pool(name="f_tmp" + sfx, bufs=16) as f_tmp, \
                     tc.tile_pool(name="f_out" + sfx, bufs=6) as f_outp, \
                     tc.tile_pool(name="f_ps" + sfx, bufs=4, space="PSUM") as f_ps:
                    for c in range(NCHUNK):
                        ps_o = f_ps.tile([128, CH], F32, tag="fps")
                        for h in range(2):
                            pr = f_in.tile([128, CH], PAY, tag="pin")
                            nc.sync.dma_start(
                                pr[:], cc2_out[4 * h:4 * (h + 1), :, 0, 2 * c:2 * (c + 1), :])
                            pi = f_in.tile([128, CH], PAY, tag="pin")
                            nc.sync.dma_start(
                                pi[:], cc2_out[4 * h:4 * (h + 1), :, 1, 2 * c:2 * (c + 1), :])

                            # Yf = P * conj(w)[k, n3]; cw stored as (cos, -sin)
                            yfr = f_tmp.tile([128, CH], MMD, tag="yf")
                            yfi = f_tmp.tile([128, CH], MMD, tag="yf")
                            for s2 in range(2):
                                sl = slice(B * s2, B * (s2 + 1))
                                p1 = f_tmp.tile([128, B], F32, tag="ftmp")
                                p2 = f_tmp.tile([128, B], F32, tag="ftmp")
                                p3 = f_tmp.tile([128, B], F32, tag="ftmp")
                                p4 = f_tmp.tile([128, B], F32, tag="ftmp")
                                nc.vector.tensor_mul(p1[:], pr[:, sl], cwr_h[h][:])
                                nc.gpsimd.tensor_mul(p2[:], pi[:, sl], cwi_h[h][:])
                                nc.vector.tensor_mul(p3[:], pi[:, sl], cwr_h[h][:])
                                nc.gpsimd.tensor_mul(p4[:], pr[:, sl], cwi_h[h][:])
                                nc.vector.tensor_add(yfr[:, sl], p1[:], p2[:])
                                nc.vector.tensor_sub(yfi[:, sl], p3[:], p4[:])

                            st = h == 0
                            sp = h == 1
                            nc.tensor.matmul(ps_o[:], _mm(aw1_blk[h][:]), _mm(yfi[:]),
                                             start=st, stop=False, skip_group_check=True)
                            nc.tensor.matmul(ps_o[:], _mm(aw2_blk[h][:]), _mm(yfr[:]),
                                             start=False, stop=sp, skip_group_check=True)

                        o = f_outp.tile([128, CH], F32, tag="fout")
                        nc.scalar.copy(o[:], ps_o[:])
                        nc.sync.dma_start(y_out[:, c * CH:(c + 1) * CH], o[:])

            for rep in range(reps):
                bi = rep % nbuf
                emit(rep, cc1_ins[bi], cc1_outs[bi], cc2_ins[bi], cc2_outs[bi])

    nc.compile()
    return nc


_NC = None
_TABLES = None


def _tables():
    global _TABLES
    if _TABLES is None:
        f16 = mybir.dt.np(F16)
        k = np.arange(B)
        D = np.exp(-2j * np.pi * np.outer(k, k) / B)
        T2 = np.exp(-2j * np.pi * np.outer(k, k) / R)
        s = 1.0 / (2.0 * M)
        CW = np.exp(-2j * np.pi * np.outer(k, k) / M)  # [k, n3], (cos, -sin)
        uws = []
        for c in range(W):
            k1 = 32 * c + np.arange(32)
            th_u = 2.0 * np.pi * np.outer(np.arange(B), k1) / 65536.0  # [n2, k1l]
            th_w = 2.0 * np.pi * np.outer(np.arange(B), k1) / float(M)  # [n3, k1l]
            def fold(m):  # [256, 32] -> [128, 64] with col j = half*32 + k1l
                out = np.empty((128, 64), np.float32)
                out[:, :32] = m[:128, :]
                out[:, 32:] = m[128:, :]
                return np.ascontiguousarray(out)
            ur = fold(np.cos(th_u)); ui = fold(np.sin(th_u))
            uws.append(dict(
                ur=ur, ui=ui,
                sur=np.ascontiguousarray(s * ur), sui=np.ascontiguousarray(s * ui),
                wr=fold(np.cos(th_w)), wi=fold(np.sin(th_w)),
            ))
        _TABLES = dict(
            dr=np.ascontiguousarray(D.real.astype(np.float32)),
            di=np.ascontiguousarray(D.imag.astype(np.float32)),
            ndi=np.ascontiguousarray(-D.imag.astype(np.float32)),
            t2r=np.ascontiguousarray(T2.real.astype(np.float32)),
            t2i=np.ascontiguousarray(T2.imag.astype(np.float32)),
            aw1=np.ascontiguousarray(D.real[:, :128].astype(np.float32)),
            aw2=np.ascontiguousarray(-D.imag[:, :128].astype(np.float32)),
            cwr=np.ascontiguousarray(CW.real.astype(f16)),
            cwi=np.ascontiguousarray(CW.imag.astype(f16)),
            uws=uws,
        )
    return _TABLES


def _in_map(a_c, x_c, tb, c):
    return dict(
        a_c=np.ascontiguousarray(a_c),
        x_c=np.ascontiguousarray(x_c),
        dr=tb["dr"], di=tb["di"], ndi=tb["ndi"],
        t2r=tb["t2r"], t2i=tb["t2i"],
        aw1=tb["aw1"], aw2=tb["aw2"],
        cwr=tb["cwr"], cwi=tb["cwi"],
        **tb["uws"][c],
    )


def kernel(a, x, _want_trace=False, **_unused):
    global _NC
    a = np.asarray(a, dtype=np.float32)
    x = np.asarray(x, dtype=np.float32)
    tb = _tables()
    if _NC is None:
        _NC = build_nc()

    a3 = a.reshape(128, W, RL)
    x3 = x.reshape(128, W, RL)
    in_maps = []
    for c in range(W):
        in_maps.append(_in_map(a3[:, c, :], x3[:, c, :], tb, c))
    res = run_bass_kernel_spmd(_NC, in_maps, core_ids=list(range(W)),
                               trace=_want_trace)
    full = np.empty((128, R), dtype=np.float32)
    for c in range(W):
        full[:, c * RL:(c + 1) * RL] = res.results[c]["y_c"]
    out = full.reshape(-1)
    if _want_trace:
        return out, res
    return out
